# revision 1
# baseline (speedup 1.0000x reference)
"""Trainium2 Bass kernel for nn_EntropyLM (wavelet-coeff mixer + chunked MHA + output proj).

Strategy: data-parallel over the 16 independent (batch x chunk) blocks, 2 per
NeuronCore.  All matmuls run in bf16 on the PE with fp32 PSUM accumulation;
layernorm / softmax statistics are computed in fp32.

Layout convention per chunk (CHUNK=1024 tokens, H=1024 features):
  * Linear layers contract over features, so the activation operand of each
    matmul must be feature-major ("T" tensors: [feat_part, token_free]).
  * LN / softmax reductions run along the free axis, so those stages use
    token-major tensors ([token_part, feat_free]).
  * Attention scores are computed directly transposed (ST = K @ Q^T, i.e.
    [k_part, q_free]); exp(ST) is then exactly the lhsT operand that the
    PV matmul needs, which avoids any on-chip transpose of the score matrix.
    The softmax denominator is computed with a ones-vector matmul (partition
    reduction on the PE) and applied per-partition after PV.
  * Orientation changes of bf16 activations go through the DMA xbar
    transpose engine (dma_start_transpose), never through the PE.
"""

import numpy as np
import ml_dtypes

B, S, H, G, W = 4, 4096, 1024, 256, 8
CHUNK = 1024
NUM_HEADS = 4
HD = H // NUM_HEADS          # 256 per-head dim
HM = H // 2                  # 512 mixer hidden
N_CHUNKS = B * (S // CHUNK)  # 16 independent chunks
N_CORES = 8
CPC = N_CHUNKS // N_CORES    # 2 chunks per core
NT = CHUNK // 128            # 8 token tiles
KH = H // 128                # 8 feature tiles (H)
KM = HM // 128               # 4 feature tiles (HM)
EPS = 1e-5
BF16 = ml_dtypes.bfloat16

_COMPILED = None


def _build(debug=False):
    import concourse.bass as bass  # noqa: F401
    import concourse.tile as tile
    from concourse import bacc, mybir

    bf = mybir.dt.bfloat16
    fp16 = mybir.dt.float16
    f32 = mybir.dt.float32
    Alu = mybir.AluOpType
    Act = mybir.ActivationFunctionType

    nc = bacc.Bacc("TRN2", target_bir_lowering=False, debug=False,
                   enable_asserts=True, num_devices=N_CORES)

    # ---- DRAM tensors (per-core views; same NEFF on all 8 cores) ----
    xt = nc.dram_tensor("xt", [CPC, H, CHUNK], bf, kind="ExternalInput")
    kernT = nc.dram_tensor("kernt", [H, W], bf, kind="ExternalInput")
    w1a = nc.dram_tensor("w1a", [W + 1, HM], bf, kind="ExternalInput")
    gln = nc.dram_tensor("gln", [128, KM], f32, kind="ExternalInput")
    bln = nc.dram_tensor("bln", [128, KM], f32, kind="ExternalInput")
    w2 = nc.dram_tensor("w2", [HM, H], bf, kind="ExternalInput")
    b2c = nc.dram_tensor("b2c", [128, KH], f32, kind="ExternalInput")
    wq = nc.dram_tensor("wq", [H, H], bf, kind="ExternalInput")
    wk = nc.dram_tensor("wk", [H, H], bf, kind="ExternalInput")
    wv = nc.dram_tensor("wv", [H, H], bf, kind="ExternalInput")
    wo = nc.dram_tensor("wo", [H, H], bf, kind="ExternalInput")
    gw = nc.dram_tensor("gw", [H, G], bf, kind="ExternalInput")
    bw = nc.dram_tensor("bw", [128, G], f32, kind="ExternalInput")
    y = nc.dram_tensor("y", [CPC, CHUNK, G], f32, kind="ExternalOutput")
    dbg = {}
    if debug:
        for nm, shp, dt in [
            ("dcoef", [W + 1, CHUNK], bf),
            ("dhidT", [128, KM, CHUNK], bf), ("dmixT", [128, KH, CHUNK], bf),
            ("dmixN", [128, NT, H], bf), ("dqT", [128, KH, CHUNK], bf),
            ("dkT", [128, KH, CHUNK], bf), ("dvN", [128, NT, H], fp16),
            ("det", [128, KH, CHUNK], fp16), ("docat", [128, NT, H], bf),
            ("dres", [128, NT, H], bf), ("dz", [128, NT, H], bf),
            ("dzT", [128, KH, CHUNK], bf), ("dsq", [128, NT], f32),
        ]:
            dbg[nm] = nc.dram_tensor(nm, shp, dt, kind="ExternalOutput")

    with tile.TileContext(nc) as tc:
        with (
            tc.tile_pool(name="wp", bufs=1) as wp,
            tc.tile_pool(name="ws", bufs=1) as ws,
            tc.tile_pool(name="sm", bufs=2) as sm,
            tc.tile_pool(name="ps", bufs=3, space="PSUM") as ps,
            tc.tile_pool(name="ps2", bufs=2, space="PSUM") as ps2,
        ):
            # ---------- persistent weights ----------
            kt_sb = wp.tile([128, KH, W], bf, tag="ktw")
            nc.sync.dma_start(kt_sb[:], kernT.ap().rearrange("(i p) w -> p i w", p=128))
            w1a_sb = wp.tile([W + 1, HM], bf, tag="w1a")
            nc.sync.dma_start(w1a_sb[:], w1a.ap())
            gln_sb = wp.tile([128, KM], f32, tag="gln")
            nc.sync.dma_start(gln_sb[:], gln.ap())
            bln_sb = wp.tile([128, KM], f32, tag="bln")
            nc.sync.dma_start(bln_sb[:], bln.ap())
            b2_sb = wp.tile([128, KH], f32, tag="b2")
            nc.sync.dma_start(b2_sb[:], b2c.ap())
            gw_sb = wp.tile([128, KH, G], bf, tag="gw")
            nc.sync.dma_start(gw_sb[:], gw.ap().rearrange("(i p) g -> p i g", p=128))
            bw_sb = wp.tile([128, G], f32, tag="bw")
            nc.sync.dma_start(bw_sb[:], bw.ap())
            ones_sb = wp.tile([128, 1], fp16, tag="ones")
            nc.vector.memset(ones_sb[:], 1.0)
            eps_sb = wp.tile([128, 1], f32, tag="eps")
            nc.vector.memset(eps_sb[:], EPS)

            def stream_w(src):
                dst = ws.tile([128, KH, H], bf, tag="wstream", bufs=2, name="wst")
                nc.sync.dma_start(dst[:], src.ap().rearrange("(i p) m -> p i m", p=128))
                return dst

            # ---------- stage 1 (both chunks up front): wavelet coeffs ----------
            # Running chunk 1's input load + tiny coeff matmuls during chunk 0's
            # mixer window removes the chunk-boundary DMA stall.
            coefs = []
            for c in range(CPC):
                xts = ws.tile([128, KH, CHUNK], bf, tag="xts_et", bufs=2)
                for ii in range(2):
                    nc.sync.dma_start(
                        xts[:, ii * 4:(ii + 1) * 4, :],
                        xt.ap()[c, ii * 512:(ii + 1) * 512, :].rearrange(
                            "(i p) t -> p i t", p=128))
                coef = ws.tile([W + 1, CHUNK], bf, tag="coef", bufs=2)
                # row W is the constant 1.0 bias row for the folded mix_b1
                nc.gpsimd.memset(coef[:, :], 1.0)
                for n in range(2):
                    cps = ps.tile([128, 512], f32, tag="mm")
                    for i in range(KH):
                        nc.tensor.matmul(cps[:W, :], kt_sb[:, i, :],
                                         xts[:, i, n * 512:(n + 1) * 512],
                                         start=(i == 0), stop=(i == KH - 1))
                    nc.scalar.copy(coef[:W, n * 512:(n + 1) * 512], cps[:W, :])
                coefs.append(coef)

            for c in range(CPC):
                coef = coefs[c]
                w2s = ws.tile([128, KM, H], bf, tag="wstream", bufs=2, name="w2s")
                nc.sync.dma_start(w2s[:], w2.ap().rearrange("(i p) m -> p i m", p=128))
                wq_sb = stream_w(wq)
                wk_sb = stream_w(wk)
                if debug and c == 0:
                    nc.sync.dma_start(dbg["dcoef"].ap(), coef[:])
                # ---------- stage 2: mixer hidden + LN + gelu -> hidT ----------
                # z1 = (pre-m)*inv in token-major (stats per-partition), then
                # transpose; gamma/beta + gelu applied feature-major where
                # they are per-partition -> one fused TS + in-place gelu.
                hidT = ws.tile([128, KM, CHUNK], bf, tag="hidT")
                for t in range(NT):
                    hps = ps.tile([128, 512], f32, tag="mm")
                    nc.tensor.matmul(hps[:], coef[:, t * 128:(t + 1) * 128],
                                     w1a_sb[:], start=True, stop=True)
                    st6 = sm.tile([128, 6], f32, tag="st6")
                    nc.vector.bn_stats(st6[:], hps[:])
                    mv = sm.tile([128, 2], f32, tag="mv")
                    nc.vector.bn_aggr(mv[:], st6[:])
                    sq = sm.tile([128, 1], f32, tag="sq")
                    nc.scalar.activation(sq[:], mv[:, 1:2], Act.Sqrt, bias=eps_sb[:])
                    iv = sm.tile([128, 1], f32, tag="iv")
                    nc.vector.reciprocal(iv[:], sq[:])
                    tmp = sm.tile([128, HM], bf, tag="mtmp")
                    nc.vector.tensor_scalar(tmp[:], hps[:],
                                            mv[:, 0:1], iv[:],
                                            op0=Alu.subtract, op1=Alu.mult)
                    nc.sync.dma_start_transpose(hidT[:, :, t * 128:(t + 1) * 128],
                                                tmp[:])
                for nh in range(2):
                    for ki in range(KM):
                        sl = hidT[:, ki, nh * 512:(nh + 1) * 512]
                        nc.vector.tensor_scalar(sl, sl,
                                                gln_sb[:, ki:ki + 1], bln_sb[:, ki:ki + 1],
                                                op0=Alu.mult, op1=Alu.add)
                        nc.scalar.activation(sl, sl, Act.Gelu)

                if debug and c == 0:
                    nc.sync.dma_start(dbg["dhidT"].ap(), hidT[:])
                # ---------- stage 3: mixedT (+b2) and mixed_nat ----------
                mixT = ws.tile([128, KH, CHUNK], bf, tag="mixT_z", bufs=2)
                for n in range(2):
                    for m in range(KH):
                        mps = ps.tile([128, 512], f32, tag="mm")
                        for ki in range(KM):
                            nc.tensor.matmul(mps[:], w2s[:, ki, m * 128:(m + 1) * 128],
                                             hidT[:, ki, n * 512:(n + 1) * 512],
                                             start=(ki == 0), stop=(ki == KM - 1))
                        nc.vector.tensor_scalar(mixT[:, m, n * 512:(n + 1) * 512], mps[:],
                                                b2_sb[:, m:m + 1], None, op0=Alu.add)
                mixN = ws.tile([128, NT, H], bf, tag="mixN")
                for m in range(KH):
                    nc.sync.dma_start_transpose(mixN[:, :, m * 128:(m + 1) * 128],
                                                mixT[:, m, :])

                if debug and c == 0:
                    nc.sync.dma_start(dbg["dmixT"].ap(), mixT[:])
                    nc.sync.dma_start(dbg["dmixN"].ap(), mixN[:])
                # ---------- stage 4: qT, kT, v ----------
                qT = ws.tile([128, KH, CHUNK], bf, tag="qT_otc")
                kT = ws.tile([128, KH, CHUNK], bf, tag="kT_zT")
                for (dst, wsb, on_act) in ((qT, wq_sb, True), (kT, wk_sb, False)):
                    for n in range(2):
                        for m in range(KH):
                            qps = ps.tile([128, 512], f32, tag="mm")
                            for ki in range(KH):
                                nc.tensor.matmul(qps[:], wsb[:, ki, m * 128:(m + 1) * 128],
                                                 mixT[:, ki, n * 512:(n + 1) * 512],
                                                 start=(ki == 0), stop=(ki == KH - 1))
                            if on_act:
                                nc.scalar.copy(dst[:, m, n * 512:(n + 1) * 512], qps[:])
                            else:
                                nc.vector.tensor_copy(dst[:, m, n * 512:(n + 1) * 512], qps[:])
                wv_sb = stream_w(wv)
                vN = ws.tile([128, NT, H], fp16, tag="hp_v")
                for t in range(NT):
                    for n in range(2):
                        vps = ps.tile([128, 512], f32, tag="mm")
                        for ki in range(KH):
                            nc.tensor.matmul(vps[:], mixT[:, ki, t * 128:(t + 1) * 128],
                                             wv_sb[:, ki, n * 512:(n + 1) * 512],
                                             start=(ki == 0), stop=(ki == KH - 1))
                        nc.scalar.copy(vN[:, t, n * 512:(n + 1) * 512], vps[:])

                if debug and c == 0:
                    nc.sync.dma_start(dbg["dqT"].ap(), qT[:])
                    nc.sync.dma_start(dbg["dkT"].ap(), kT[:])
                    nc.sync.dma_start(dbg["dvN"].ap(), vN[:])
                wo_sb = stream_w(wo)
                # ---------- stage 5: attention ----------
                ocat = ws.tile([128, NT, H], bf, tag="hidT_oc_res")
                if debug and c == 0:
                    dsq_sb = sm.tile([128, NT], f32, tag="dsq")
                for h in range(NUM_HEADS):
                    et = ws.tile([128, KH, CHUNK], fp16, tag="xts_et", bufs=2)
                    for kt in range(NT):
                        stp = ps2.tile([128, CHUNK], f32, tag="st")
                        for qn in range(2):
                            for dk in range(2):
                                nc.tensor.matmul(
                                    stp[:, qn * 512:(qn + 1) * 512],
                                    kT[:, 2 * h + dk, kt * 128:(kt + 1) * 128],
                                    qT[:, 2 * h + dk, qn * 512:(qn + 1) * 512],
                                    start=(dk == 0), stop=(dk == 1))
                        # exp(score/sqrt(hd)); values are O(1e-1) so no max-sub needed
                        nc.scalar.activation(et[:, kt, :], stp[:], Act.Exp,
                                             scale=float(HD ** -0.5))
                    for qt in range(NT):
                        ovp = ps.tile([128, 512], f32, tag="mm")
                        for kt in range(NT):
                            # O_unnorm[q, d] accumulation; the extra N=1 matmul
                            # with a ones column gives s[q] = sum_k exp in the
                            # same [q_part, 1] orientation the normalization
                            # needs (same lhsT -> weight load is reused).
                            nc.tensor.matmul(ovp[:, :HD], et[:, kt, qt * 128:(qt + 1) * 128],
                                             vN[:, kt, h * HD:(h + 1) * HD],
                                             start=(kt == 0), stop=(kt == NT - 1))
                            # start=False even at kt==0: start=True clears the
                            # whole PSUM bank and would wipe the V-matmul's
                            # kt==0 contribution.  The bank-clear from the
                            # V-matmul above leaves this column's has_written
                            # bits 0, so kt==0 overwrites (not accumulates).
                            nc.tensor.matmul(ovp[:, HD:HD + 1],
                                             et[:, kt, qt * 128:(qt + 1) * 128],
                                             ones_sb[:],
                                             start=False, stop=(kt == NT - 1),
                                             skip_group_check=True)
                        rq = sm.tile([128, 1], f32, tag="rq")
                        if debug and c == 0 and h == NUM_HEADS - 1:
                            nc.vector.tensor_copy(dsq_sb[:, qt:qt + 1], ovp[:, HD:HD + 1])
                        nc.vector.reciprocal(rq[:], ovp[:, HD:HD + 1])
                        nc.vector.tensor_scalar(ocat[:, qt, h * HD:(h + 1) * HD],
                                                ovp[:, :HD], rq[:], None,
                                                op0=Alu.mult)
                otc = ws.tile([128, KH, CHUNK], bf, tag="qT_otc")
                for qt in range(NT):
                    nc.sync.dma_start_transpose(otc[:, :, qt * 128:(qt + 1) * 128],
                                                ocat[:, qt, :])

                if debug and c == 0:
                    nc.sync.dma_start(dbg["det"].ap(), et[:])
                    nc.sync.dma_start(dbg["docat"].ap(), ocat[:])
                    nc.sync.dma_start(dbg["dsq"].ap(), dsq_sb[:])
                # ---------- stage 6: wo proj + residual + out LN ----------
                res = ws.tile([128, NT, H], bf, tag="hidT_oc_res")
                z = ws.tile([128, NT, H], bf, tag="mixT_z", bufs=2)
                zT = ws.tile([128, KH, CHUNK], bf, tag="kT_zT")
                for t in range(NT):
                    for n in range(2):
                        ops_ = ps.tile([128, 512], f32, tag="mm")
                        for fi in range(KH):
                            nc.tensor.matmul(ops_[:], otc[:, fi, t * 128:(t + 1) * 128],
                                             wo_sb[:, fi, n * 512:(n + 1) * 512],
                                             start=(fi == 0), stop=(fi == KH - 1))
                        nc.vector.tensor_add(res[:, t, n * 512:(n + 1) * 512], ops_[:],
                                             mixN[:, t, n * 512:(n + 1) * 512])
                    st6 = sm.tile([128, 2, 6], f32, tag="st6b")
                    for half in range(2):
                        nc.vector.bn_stats(st6[:, half, :],
                                           res[:, t, half * 512:(half + 1) * 512])
                    mv = sm.tile([128, 2], f32, tag="mv")
                    nc.vector.bn_aggr(mv[:], st6[:])
                    sq = sm.tile([128, 1], f32, tag="sq")
                    nc.scalar.activation(sq[:], mv[:, 1:2], Act.Sqrt, bias=eps_sb[:])
                    iv = sm.tile([128, 1], f32, tag="iv")
                    nc.vector.reciprocal(iv[:], sq[:])
                    nc.vector.tensor_scalar(z[:, t, :], res[:, t, :],
                                            mv[:, 0:1], iv[:],
                                            op0=Alu.subtract, op1=Alu.mult)
                    nc.sync.dma_start_transpose(zT[:, :, t * 128:(t + 1) * 128],
                                                z[:, t, :])

                if debug and c == 0:
                    nc.sync.dma_start(dbg["dres"].ap(), res[:])
                    nc.sync.dma_start(dbg["dz"].ap(), z[:])
                    nc.sync.dma_start(dbg["dzT"].ap(), zT[:])
                # ---------- stage 7: output projection ----------
                ych = ws.tile([128, NT, G], f32, tag="ych", bufs=1)
                for t in range(NT):
                    yps = ps.tile([128, 512], f32, tag="mm")
                    for fi in range(KH):
                        nc.tensor.matmul(yps[:, :G], zT[:, fi, t * 128:(t + 1) * 128],
                                         gw_sb[:, fi, :],
                                         start=(fi == 0), stop=(fi == KH - 1))
                    nc.vector.tensor_add(ych[:, t, :], yps[:, :G], bw_sb[:])
                for hh in range(2):
                    nc.sync.dma_start(
                        y.ap()[c, hh * 512:(hh + 1) * 512, :].rearrange(
                            "(t p) g -> p t g", p=128),
                        ych[:, hh * 4:(hh + 1) * 4, :])

    nc.compile()
    return nc


def _get_compiled():
    global _COMPILED
    if _COMPILED is None:
        _COMPILED = _build()
    return _COMPILED


def _prep_inputs(inputs):
    f32 = np.float32

    def a(name):
        return np.asarray(inputs[name], dtype=f32)

    x = a("x")
    mw = a("mother_wavelets")
    scales = a("scales")
    norm = np.sqrt(np.sum(mw ** 2, axis=2, keepdims=True))
    kern = (mw / np.maximum(norm, 1e-12)) * (1.0 / (1.0 + np.exp(-scales)))
    kern = kern[0, :, :, 0]                      # (W, H)
    kernT = np.ascontiguousarray(kern.T).astype(BF16)

    w1a = np.concatenate([a("mix_w1"), a("mix_b1")[None, :]], axis=0).astype(BF16)
    gln = np.ascontiguousarray(a("mix_ln_g").reshape(KM, 128).T).astype(f32)
    bln = np.ascontiguousarray(a("mix_ln_b").reshape(KM, 128).T).astype(f32)
    w2 = a("mix_w2").astype(BF16)
    b2c = np.ascontiguousarray(a("mix_b2").reshape(KH, 128).T).astype(f32)
    gw = (a("out_ln_g")[:, None] * a("out_w")).astype(BF16)
    bw_vec = a("out_ln_b") @ a("out_w") + a("out_b")
    bw = np.tile(bw_vec[None, :], (128, 1)).astype(f32)

    shared = {
        "kernt": kernT, "w1a": w1a, "gln": gln, "bln": bln, "w2": w2,
        "b2c": b2c, "wq": a("wq").astype(BF16), "wk": a("wk").astype(BF16),
        "wv": a("wv").astype(BF16), "wo": a("wo").astype(BF16),
        "gw": gw, "bw": bw,
    }

    xc = x.reshape(N_CHUNKS, CHUNK, H)
    xt_all = np.ascontiguousarray(xc.transpose(0, 2, 1)).astype(BF16)  # (16, H, CHUNK)
    in_maps = []
    for core in range(N_CORES):
        m = dict(shared)
        m["xt"] = np.ascontiguousarray(xt_all[core * CPC:(core + 1) * CPC])
        in_maps.append(m)
    return in_maps


def kernel(**inputs) -> np.ndarray:
    from concourse.bass_utils import run_bass_kernel_spmd

    nc = _get_compiled()
    in_maps = _prep_inputs(inputs)
    res = run_bass_kernel_spmd(nc, in_maps, core_ids=list(range(N_CORES)))
    out = np.concatenate([r["y"] for r in res.results], axis=0)  # (16, CHUNK, G)
    return out.reshape(B, S, G).astype(np.float32)



# revision 16
# speedup vs baseline: 1.5541x; 1.5541x over previous
"""Trainium2 Bass kernel for nn_EntropyLM (wavelet-coeff mixer + chunked MHA + output proj).

Data-parallel: 16 (batch x chunk) blocks, 2 per NeuronCore.  Heavy matmuls
(q/k/v projections, attention scores, PV, wo) run fp8e4m3 with DoubleRow perf
mode (256-deep contraction / instr at 0.5 cycles/row); trunk-critical matmuls
(coeff, mixer w1/w2, output projection) stay bf16.  All weights are SBUF-
resident.  Scaling plan (validated numerically in precheck.py):
  wq8/wk8/wv8/wo8 = fp8(16*w);  qT8/kT8 = fp8(psum) = 16*q / 16*k
  scores_psum = 256*s -> exp scale = HD^-0.5/256;  et8 = fp8(exp)
  vN8 = fp8(psum/4) = 4*v;  ones8 = 0.5 -> denominator D = 0.5*sum(et)
  ocat = pv_psum * recip(D) = 8*o (bf16);  otc8 = fp8(8*o)
  wo_psum = 128*(o@wo) -> res = wo_psum*(1/128) + mixed
Output LN folds 1/sigma into the final projection:  z = res - mean (bf16),
y = (z @ gw) * rsqrt(var+eps) + bw  -- per-token scalar applied at the
PSUM->SBUF copy, so the out-LN needs one batched Sqrt per chunk (minimal
activation-table switching against the attention exp ops).
"""

import numpy as np
import ml_dtypes

B, S, H, G, W = 4, 4096, 1024, 256, 8
CHUNK = 1024
NUM_HEADS = 4
HD = H // NUM_HEADS          # 256 per-head dim
HM = H // 2                  # 512 mixer hidden
N_CHUNKS = B * (S // CHUNK)  # 16 independent chunks
N_CORES = 8
CPC = N_CHUNKS // N_CORES    # 2 chunks per core
NT = CHUNK // 128            # 8 token tiles per chunk
KH = H // 128                # 8 feature tiles (H)
KM = HM // 128               # 4 feature tiles (HM)
EPS = 1e-5
BF16 = ml_dtypes.bfloat16
F8 = ml_dtypes.float8_e4m3   # TRN fp8e4: max normal 240 (matches ml_dtypes e4m3)

_COMPILED = None


def _build(debug=False):
    import concourse.bass as bass  # noqa: F401
    import concourse.tile as tile
    from concourse import bacc, mybir

    bf = mybir.dt.bfloat16
    f8 = mybir.dt.float8e4
    f32 = mybir.dt.float32
    Alu = mybir.AluOpType
    Act = mybir.ActivationFunctionType
    DR = mybir.MatmulPerfMode.DoubleRow

    nc = bacc.Bacc("TRN2", target_bir_lowering=False, debug=False,
                   enable_asserts=True, num_devices=N_CORES)

    # ---- DRAM tensors (per-core views; same NEFF on all 8 cores) ----
    xt = nc.dram_tensor("xt", [CPC, KH, 128, CHUNK], bf, kind="ExternalInput")
    kernT = nc.dram_tensor("kernt", [H, W], bf, kind="ExternalInput")
    w1a = nc.dram_tensor("w1a", [W + 1, HM], bf, kind="ExternalInput")
    gln = nc.dram_tensor("gln", [128, KM], f32, kind="ExternalInput")
    bln = nc.dram_tensor("bln", [128, KM], f32, kind="ExternalInput")
    w2 = nc.dram_tensor("w2", [HM, H], bf, kind="ExternalInput")
    b2c = nc.dram_tensor("b2c", [128, KH], f32, kind="ExternalInput")
    wq8 = nc.dram_tensor("wq8", [H, H], f8, kind="ExternalInput")
    wk8 = nc.dram_tensor("wk8", [H, H], f8, kind="ExternalInput")
    wv8 = nc.dram_tensor("wv8", [H, H], f8, kind="ExternalInput")
    wo8 = nc.dram_tensor("wo8", [H, H], f8, kind="ExternalInput")
    gw = nc.dram_tensor("gw", [H, G], bf, kind="ExternalInput")
    bw = nc.dram_tensor("bw", [128, G], f32, kind="ExternalInput")
    y = nc.dram_tensor("y", [CPC, CHUNK, G], f32, kind="ExternalOutput")
    dbg = {}
    if debug:
        for nm, shp, dt in [
            ("dcoef", [W + 1, CHUNK], bf),
            ("dhidT", [128, KM, CHUNK], bf),
            ("dmix8", [128, KH, CHUNK], f8),
            ("dmixN", [128, NT, H], bf),
            ("dq8", [128, KH, CHUNK], f8),
            ("dk8", [128, KH, CHUNK], f8),
            ("dv8", [128, NT, H], f8),
            ("det8", [128, NT, CHUNK], f8),
            ("docat", [128, NT, H], bf),
            ("dres", [128, NT, H], bf),
        ]:
            dbg[nm] = nc.dram_tensor(nm, shp, dt, kind="ExternalOutput")

    with tile.TileContext(nc) as tc:
        with (
            tc.tile_pool(name="wp", bufs=1) as wp,
            tc.tile_pool(name="ws", bufs=1) as ws,
            tc.tile_pool(name="sm", bufs=3) as sm,
            tc.tile_pool(name="ps", bufs=4, space="PSUM") as ps,
            tc.tile_pool(name="ps2", bufs=2, space="PSUM") as ps2,
        ):
            # ---------- persistent weights (loads staged to not block x) ----
            kt_sb = wp.tile([128, KH, W], bf, tag="ktw")
            nc.sync.dma_start(kt_sb[:], kernT.ap().rearrange("(i p) w -> p i w", p=128))
            ones8_sb = wp.tile([128, 2, 1], f8, tag="ones8")
            nc.vector.memset(ones8_sb[:], 0.5)
            eps_sb = wp.tile([128, 1], f32, tag="eps")
            nc.vector.memset(eps_sb[:], EPS)
            w1a_sb = wp.tile([W + 1, HM], bf, tag="w1a")
            gln_sb = wp.tile([128, KM], f32, tag="gln")
            bln_sb = wp.tile([128, KM], f32, tag="bln")
            b2_sb = wp.tile([128, KH], f32, tag="b2")
            w2_sb = wp.tile([128, KM, H], bf, tag="w2w")
            wq_sb = wp.tile([128, KH, H], f8, tag="wqw")
            wk_sb = wp.tile([128, KH, H], f8, tag="wkw")
            wv_sb = wp.tile([128, KH, H], f8, tag="wvw")
            wo_sb = wp.tile([128, KH, H], f8, tag="wow")
            gw_sb = wp.tile([128, KH, G], bf, tag="gw")
            bw_sb = wp.tile([128, G], f32, tag="bw")

            def load_w8(dst, src):
                nc.sync.dma_start(dst[:], src.ap().rearrange("(i p) m -> p i m", p=128))

            def stage1(c):
                """wavelet coeffs for chunk c -> coef (bf16 [W+1, CHUNK])"""
                coef = ws.tile([W + 1, CHUNK], bf, tag="coef", bufs=2)
                nc.gpsimd.memset(coef[:, :], 1.0)  # row W = folded-bias ones row
                cps = ps2.tile([128, CHUNK], f32, tag="st")
                for ki in range(KH):
                    xki = ws.tile([128, CHUNK], bf, tag="xki", bufs=2)
                    nc.sync.dma_start(xki[:], xt.ap()[c, ki])
                    for n in range(2):
                        # each 512-token half accumulates in its own PSUM bank
                        nc.tensor.matmul(cps[:W, n * 512:(n + 1) * 512],
                                         kt_sb[:, ki, :],
                                         xki[:, n * 512:(n + 1) * 512],
                                         start=(ki == 0),
                                         stop=(ki == KH - 1))
                nc.scalar.copy(coef[:W, :], cps[:W, :])
                return coef

            def stage2(c, coef, hidT):
                """mixer hidden + LN + gelu -> hidT (feature-major bf16)"""
                for t in range(NT):
                    hps = ps.tile([128, 512], f32, tag="mm")
                    nc.tensor.matmul(hps[:], coef[:, t * 128:(t + 1) * 128],
                                     w1a_sb[:], start=True, stop=True)
                    st6 = sm.tile([128, 6], f32, tag="st6")
                    nc.vector.bn_stats(st6[:], hps[:])
                    mv = sm.tile([128, 2], f32, tag="mv")
                    nc.vector.bn_aggr(mv[:], st6[:])
                    sq = sm.tile([128, 1], f32, tag="sq")
                    nc.scalar.activation(sq[:], mv[:, 1:2], Act.Sqrt, bias=eps_sb[:])
                    iv = sm.tile([128, 1], f32, tag="iv")
                    nc.vector.reciprocal(iv[:], sq[:])
                    tmp = sm.tile([128, HM], bf, tag="mtmp")
                    nc.vector.tensor_scalar(tmp[:], hps[:], mv[:, 0:1], iv[:],
                                            op0=Alu.subtract, op1=Alu.mult)
                    nc.sync.dma_start_transpose(hidT[:, :, t * 128:(t + 1) * 128],
                                                tmp[:])
                for ki in range(KM):
                    sl = hidT[:, ki, :]
                    nc.gpsimd.tensor_scalar(sl, sl,
                                            gln_sb[:, ki:ki + 1], bln_sb[:, ki:ki + 1],
                                            op0=Alu.mult, op1=Alu.add)
                for ki in range(KM):
                    nc.scalar.activation(hidT[:, ki, :], hidT[:, ki, :], Act.Gelu)

            def stage3(c, hidT, mix8, mixN):
                """w2 matmul -> mixT8 (fp8 qkv operand) + mixN (bf16 residual)"""
                for m in range(KH):
                    mrot = ws.tile([128, CHUNK], bf, tag="mrot", bufs=2)
                    for n in range(2):
                        mps = ps.tile([128, 512], f32, tag="mm")
                        for ki in range(KM):
                            nc.tensor.matmul(mps[:], w2_sb[:, ki, m * 128:(m + 1) * 128],
                                             hidT[:, ki, n * 512:(n + 1) * 512],
                                             start=(ki == 0), stop=(ki == KM - 1))
                        nc.vector.tensor_scalar(mrot[:, n * 512:(n + 1) * 512], mps[:],
                                                b2_sb[:, m:m + 1], None, op0=Alu.add)
                        nc.scalar.activation(mix8[:, m, n * 512:(n + 1) * 512], mps[:],
                                             Act.Identity, bias=b2_sb[:, m:m + 1])
                    nc.sync.dma_start_transpose(mixN[:, :, m * 128:(m + 1) * 128],
                                                mrot[:])

            def stage4(c, mix8, q8, k8, v8):
                """fp8 DoubleRow q/k (feature-major out) + v (token-major out).
                q/k interleaved per m so early heads' tiles land first."""
                for m in range(KH):
                    for (dst, wsb, eng) in ((q8, wq_sb, "act"), (k8, wk_sb, "dve")):
                        for n in range(2):
                            qps = ps.tile([128, 512], f32, tag="mm")
                            for j in range(KH // 2):
                                nc.tensor.matmul(
                                    qps[:], wsb[:, 2 * j:2 * j + 2, m * 128:(m + 1) * 128],
                                    mix8[:, 2 * j:2 * j + 2, n * 512:(n + 1) * 512],
                                    start=(j == 0), stop=(j == KH // 2 - 1),
                                    perf_mode=DR)
                            if eng == "act":
                                nc.scalar.copy(dst[:, m, n * 512:(n + 1) * 512], qps[:])
                            else:
                                nc.vector.tensor_copy(dst[:, m, n * 512:(n + 1) * 512],
                                                      qps[:])
                for t in range(NT):
                    for n in range(2):
                        vps = ps.tile([128, 512], f32, tag="mm")
                        for j in range(KH // 2):
                            nc.tensor.matmul(
                                vps[:], mix8[:, 2 * j:2 * j + 2, t * 128:(t + 1) * 128],
                                wv_sb[:, 2 * j:2 * j + 2, n * 512:(n + 1) * 512],
                                start=(j == 0), stop=(j == KH // 2 - 1),
                                perf_mode=DR)
                        nc.vector.tensor_scalar(v8[:, t, n * 512:(n + 1) * 512], vps[:],
                                                0.25, None, op0=Alu.mult)

            def scores_head(c, h, q8, k8):
                """scores + exp for one head -> et8 fp8 [128, NT, CHUNK]"""
                et8 = ws.tile([128, NT, CHUNK], f8, tag="et8", bufs=2)
                for kt in range(NT):
                    stp = ps2.tile([128, CHUNK], f32, tag="st")
                    for qn in range(2):
                        nc.tensor.matmul(
                            stp[:, qn * 512:(qn + 1) * 512],
                            k8[:, 2 * h:2 * h + 2, kt * 128:(kt + 1) * 128],
                            q8[:, 2 * h:2 * h + 2, qn * 512:(qn + 1) * 512],
                            start=True, stop=True, perf_mode=DR)
                    nc.scalar.activation(et8[:, kt, :], stp[:], Act.Exp,
                                         scale=float(HD ** -0.5 / 256.0))
                return et8

            def pv_head(c, h, et8, v8, ocat):
                """PV + denominator + normalize -> ocat[:, qt, h*HD:(h+1)*HD]"""
                for qt in range(NT):
                    ovp = ps.tile([128, 512], f32, tag="mm")
                    for j in range(NT // 2):
                        nc.tensor.matmul(ovp[:, :HD],
                                         et8[:, 2 * j:2 * j + 2, qt * 128:(qt + 1) * 128],
                                         v8[:, 2 * j:2 * j + 2, h * HD:(h + 1) * HD],
                                         start=(j == 0), stop=(j == NT // 2 - 1),
                                         perf_mode=DR)
                        # denominator column in the same PSUM bank; start=False
                        # always (j==0 PV's bank-clear leaves has_written=0 so
                        # the first write overwrites, later ones accumulate).
                        nc.tensor.matmul(ovp[:, HD:HD + 1],
                                         et8[:, 2 * j:2 * j + 2, qt * 128:(qt + 1) * 128],
                                         ones8_sb[:],
                                         start=False, stop=(j == NT // 2 - 1),
                                         perf_mode=DR, skip_group_check=True)
                    rq = sm.tile([128, 1], f32, tag="rq")
                    nc.vector.reciprocal(rq[:], ovp[:, HD:HD + 1])
                    nc.vector.tensor_scalar(ocat[:, qt, h * HD:(h + 1) * HD],
                                            ovp[:, :HD], rq[:], None, op0=Alu.mult)

            def tail_p1(c, qt, ocat, mixN, res, mvA):
                """transpose o, wo matmul, residual into res, LN stats."""
                otcb = ws.tile([128, KH, 128], bf, tag="sm2b", bufs=3)
                nc.sync.dma_start_transpose(otcb[:], ocat[:, qt, :])
                otc8 = ws.tile([128, KH, 128], f8, tag="otc8", bufs=2)
                nc.gpsimd.tensor_copy(otc8[:], otcb[:])
                for n in range(2):
                    ops_ = ps.tile([128, 512], f32, tag="mm")
                    for j in range(KH // 2):
                        nc.tensor.matmul(ops_[:],
                                         otc8[:, 2 * j:2 * j + 2, :],
                                         wo_sb[:, 2 * j:2 * j + 2, n * 512:(n + 1) * 512],
                                         start=(j == 0), stop=(j == KH // 2 - 1),
                                         perf_mode=DR)
                    nc.vector.scalar_tensor_tensor(
                        res[:, qt, n * 512:(n + 1) * 512], ops_[:], 1.0 / 128.0,
                        mixN[:, qt, n * 512:(n + 1) * 512],
                        op0=Alu.mult, op1=Alu.add)
                st6 = sm.tile([128, 2, 6], f32, tag="st6b")
                for half in range(2):
                    nc.vector.bn_stats(st6[:, half, :],
                                       res[:, qt, half * 512:(half + 1) * 512])
                nc.vector.bn_aggr(mvA[:, qt, :], st6[:])

            def tail_iv(c, mvA, ivA):
                """batched rsqrt(var+eps) for the whole chunk (one Sqrt op)."""
                sqA = sm.tile([128, NT], f32, tag="sqA")
                nc.scalar.activation(sqA[:], mvA[:, :, 1], Act.Sqrt, bias=eps_sb[:])
                nc.vector.reciprocal(ivA[:], sqA[:])

            def tail_p2(c, qt, res, mvA, ivA, ych):
                """z = res - mean; y = (z @ gw) * iv + bw."""
                z = ws.tile([128, CHUNK], bf, tag="sm2b", bufs=3, name="z")
                nc.gpsimd.tensor_scalar(z[:], res[:, qt, :], mvA[:, qt, 0:1], None,
                                        op0=Alu.subtract)
                zT = ws.tile([128, KH, 128], bf, tag="sm2b", bufs=3, name="zT")
                nc.sync.dma_start_transpose(zT[:], z[:])
                yps = ps.tile([128, 512], f32, tag="mm")
                for fi in range(KH):
                    nc.tensor.matmul(yps[:, :G], zT[:, fi, :], gw_sb[:, fi, :],
                                     start=(fi == 0), stop=(fi == KH - 1))
                nc.vector.scalar_tensor_tensor(ych[:, qt, :], yps[:, :G],
                                               ivA[:, qt:qt + 1], bw_sb[:],
                                               op0=Alu.mult, op1=Alu.add)

            def store(c, ych, t0, t1):
                nc.sync.dma_start(
                    y.ap()[c, t0 * 128:t1 * 128, :].rearrange(
                        "(t p) g -> p t g", p=128),
                    ych[:, t0:t1, :])

            # =================== emission schedule ===================
            coef0 = stage1(0)
            nc.sync.dma_start(w1a_sb[:], w1a.ap())
            nc.sync.dma_start(gln_sb[:], gln.ap())
            nc.sync.dma_start(bln_sb[:], bln.ap())
            coef1 = stage1(1)
            nc.sync.dma_start(b2_sb[:], b2c.ap())
            nc.sync.dma_start(w2_sb[:], w2.ap().rearrange("(i p) m -> p i m", p=128))

            hidT0 = ws.tile([128, KM, CHUNK], bf, tag="hidT", bufs=1)
            stage2(0, coef0, hidT0)
            load_w8(wq_sb, wq8)
            load_w8(wk_sb, wk8)
            mix8_0 = ws.tile([128, KH, CHUNK], f8, tag="mix8", bufs=1)
            mixN0 = ws.tile([128, NT, H], bf, tag="mixN", bufs=2)
            stage3(0, hidT0, mix8_0, mixN0)
            load_w8(wv_sb, wv8)
            load_w8(wo_sb, wo8)
            nc.sync.dma_start(gw_sb[:], gw.ap().rearrange("(i p) g -> p i g", p=128))
            nc.sync.dma_start(bw_sb[:], bw.ap())

            q8_0 = ws.tile([128, KH, CHUNK], f8, tag="q8", bufs=1)
            k8_0 = ws.tile([128, KH, CHUNK], f8, tag="k8", bufs=1)
            v8_0 = ws.tile([128, NT, H], f8, tag="v8y", bufs=2)
            stage4(0, mix8_0, q8_0, k8_0, v8_0)
            if debug:
                nc.sync.dma_start(dbg["dcoef"].ap(), coef0[:])
                nc.sync.dma_start(dbg["dhidT"].ap(), hidT0[:])
                nc.sync.dma_start(dbg["dmix8"].ap(), mix8_0[:])
                nc.sync.dma_start(dbg["dmixN"].ap(), mixN0[:])
                nc.sync.dma_start(dbg["dq8"].ap(), q8_0[:])
                nc.sync.dma_start(dbg["dk8"].ap(), k8_0[:])
                nc.sync.dma_start(dbg["dv8"].ap(), v8_0[:])

            # --- c0 attention interleaved with c1 front stages ---
            ocat0 = ws.tile([128, NT, H], bf, tag="ocat", bufs=1)
            et_a = scores_head(0, 0, q8_0, k8_0)
            hidT1 = ws.tile([128, KM, CHUNK], bf, tag="hidT", bufs=1)
            stage2(1, coef1, hidT1)
            et_b = scores_head(0, 1, q8_0, k8_0)
            pv_head(0, 0, et_a, v8_0, ocat0)
            if debug:
                nc.sync.dma_start(dbg["det8"].ap(), et_b[:])
            mix8_1 = ws.tile([128, KH, CHUNK], f8, tag="mix8", bufs=1)
            mixN1 = ws.tile([128, NT, H], bf, tag="mixN", bufs=2)
            stage3(1, hidT1, mix8_1, mixN1)
            et_c = scores_head(0, 2, q8_0, k8_0)
            pv_head(0, 1, et_b, v8_0, ocat0)
            et_d = scores_head(0, 3, q8_0, k8_0)
            q8_1 = ws.tile([128, KH, CHUNK], f8, tag="q8", bufs=1)
            k8_1 = ws.tile([128, KH, CHUNK], f8, tag="k8", bufs=1)
            v8_1 = ws.tile([128, NT, H], f8, tag="v8y", bufs=2)
            stage4(1, mix8_1, q8_1, k8_1, v8_1)
            pv_head(0, 2, et_c, v8_0, ocat0)

            ych0 = ws.tile([128, NT, G], f32, tag="v8y", bufs=2, name="ych")
            et_e = scores_head(1, 0, q8_1, k8_1)
            pv_head(0, 3, et_d, v8_0, ocat0)
            if debug:
                nc.sync.dma_start(dbg["docat"].ap(), ocat0[:])
            et_f = scores_head(1, 1, q8_1, k8_1)

            # --- c0 tails (two passes) with c1 PV interleaved ---
            res0 = ws.tile([128, NT, H], bf, tag="res", bufs=1)
            mvA0 = sm.tile([128, NT, 2], f32, tag="mvA", bufs=2)
            ivA0 = sm.tile([128, NT], f32, tag="ivA", bufs=2)
            ocat1 = ws.tile([128, NT, H], bf, tag="ocat", bufs=1, name="ocat")
            for qt in range(4):
                tail_p1(0, qt, ocat0, mixN0, res0, mvA0)
            pv_head(1, 0, et_e, v8_1, ocat1)
            for qt in range(4, NT):
                tail_p1(0, qt, ocat0, mixN0, res0, mvA0)
            if debug:
                nc.sync.dma_start(dbg["dres"].ap(), res0[:])
            tail_iv(0, mvA0, ivA0)
            et_g = scores_head(1, 2, q8_1, k8_1)
            for qt in range(4):
                tail_p2(0, qt, res0, mvA0, ivA0, ych0)
            store(0, ych0, 0, 4)
            pv_head(1, 1, et_f, v8_1, ocat1)
            for qt in range(4, NT):
                tail_p2(0, qt, res0, mvA0, ivA0, ych0)
            store(0, ych0, 4, NT)
            et_h = scores_head(1, 3, q8_1, k8_1)
            pv_head(1, 2, et_g, v8_1, ocat1)

            # --- c1 tails ---
            res1 = ws.tile([128, NT, H], bf, tag="res", bufs=1)
            mvA1 = sm.tile([128, NT, 2], f32, tag="mvA", bufs=2)
            ivA1 = sm.tile([128, NT], f32, tag="ivA", bufs=2)
            ych1 = ws.tile([128, NT, G], f32, tag="v8y", bufs=2, name="ych")
            pv_head(1, 3, et_h, v8_1, ocat1)
            for qt in range(NT):
                tail_p1(1, qt, ocat1, mixN1, res1, mvA1)
            tail_iv(1, mvA1, ivA1)
            for qt in range(NT):
                tail_p2(1, qt, res1, mvA1, ivA1, ych1)
                if qt == 3:
                    store(1, ych1, 0, 4)
            store(1, ych1, 4, NT)

    nc.compile()
    return nc


def _get_compiled(debug=False):
    global _COMPILED
    if _COMPILED is None:
        _COMPILED = _build(debug=debug)
    return _COMPILED


def _f8c(x):
    return np.clip(x, -240.0, 240.0).astype(F8)


def _prep_inputs(inputs):
    f32 = np.float32

    def a(name):
        return np.asarray(inputs[name], dtype=f32)

    x = a("x")
    mw = a("mother_wavelets")
    scales = a("scales")
    norm = np.sqrt(np.sum(mw ** 2, axis=2, keepdims=True))
    kern = (mw / np.maximum(norm, 1e-12)) * (1.0 / (1.0 + np.exp(-scales)))
    kern = kern[0, :, :, 0]                      # (W, H)
    kernT = np.ascontiguousarray(kern.T).astype(BF16)

    w1a = np.concatenate([a("mix_w1"), a("mix_b1")[None, :]], axis=0).astype(BF16)
    gln = np.ascontiguousarray(a("mix_ln_g").reshape(KM, 128).T).astype(f32)
    bln = np.ascontiguousarray(a("mix_ln_b").reshape(KM, 128).T).astype(f32)
    w2 = a("mix_w2").astype(BF16)
    b2c = np.ascontiguousarray(a("mix_b2").reshape(KH, 128).T).astype(f32)
    gw = (a("out_ln_g")[:, None] * a("out_w")).astype(BF16)
    bw_vec = a("out_ln_b") @ a("out_w") + a("out_b")
    bw = np.tile(bw_vec[None, :], (128, 1)).astype(f32)

    shared = {
        "kernt": kernT, "w1a": w1a, "gln": gln, "bln": bln, "w2": w2,
        "b2c": b2c,
        "wq8": _f8c(16.0 * a("wq")), "wk8": _f8c(16.0 * a("wk")),
        "wv8": _f8c(16.0 * a("wv")), "wo8": _f8c(16.0 * a("wo")),
        "gw": gw, "bw": bw,
    }

    xc = x.reshape(N_CHUNKS, CHUNK, H)
    # xt[c, ki, p, t] = xc[c, t, ki*128+p]  (feature-major per 128-slice)
    xt_all = np.ascontiguousarray(
        xc.transpose(0, 2, 1).reshape(N_CHUNKS, KH, 128, CHUNK)).astype(BF16)
    in_maps = []
    for core in range(N_CORES):
        m = dict(shared)
        m["xt"] = np.ascontiguousarray(xt_all[core * CPC:(core + 1) * CPC])
        in_maps.append(m)
    return in_maps


def kernel(**inputs) -> np.ndarray:
    from concourse.bass_utils import run_bass_kernel_spmd

    nc = _get_compiled()
    in_maps = _prep_inputs(inputs)
    res = run_bass_kernel_spmd(nc, in_maps, core_ids=list(range(N_CORES)))
    out = np.concatenate([r["y"] for r in res.results], axis=0)  # (16, CHUNK, G)
    return out.reshape(B, S, G).astype(np.float32)
